# revision 1
# baseline (speedup 1.0000x reference)
"""Trainium2 Bass kernel for nn_ConceptIntergation (histogram_binning).

Reference computation:
    counts[b,s,n] = sum_k one_hot(concepts[b,s,k], 129)[..., n]  (n < 128; 128 = padding)
    out[b,s,n,d]  = counts[b,s,n] * emb_table[n,d]

Strategy (data-parallel over batch, 8 cores):
  - Each core handles B_LOC=8 batches -> 1600 (b,s) rows, output shard
    [1600, 128*64] f32 (~52 MB). The kernel is HBM-write bound; the whole
    design keeps the store stream saturated from ~10us to the end.
  - Rows are processed in 128-row blocks (s on partitions). Histogram via
    iota-compare on DVE (tensor_scalar is_equal + scalar_tensor_tensor
    accumulate), then broadcast tensor_tensor multiplies produce
    [128, 2048] chunks = counts[:,n] * emb[n,d]; each chunk is a 1 MB DMA
    store (contiguous 8 KB per partition).
  - The embedding table is loaded once as a single 32 KB row and
    replicated across partitions on-device by GpSimd partition_broadcast,
    chunk by chunk, so the first multiplies start early and no 4 MB
    replica load competes with the store stream.
"""

import numpy as np

import concourse.bass as bass
import concourse.mybir as mybir
from concourse import bacc
from concourse.tile import TileContext
from concourse.bass_utils import run_bass_kernel_spmd

B, S, K = 64, 200, 4
N, D = 128, 64
ND = N * D                      # 8192
NCORES = 8
B_LOC = B // NCORES             # 8
ROWS = B_LOC * S                # 1600 (b,s) rows per core
P = 128
NBLK = (ROWS + P - 1) // P      # 13 (12 full + 1 of 64 rows)

CH = 4                          # emb-replica/mul/store chunks per block
CW = ND // CH                   # 2048 cols per chunk (= 16 n-rows), 1 MB stores
NCH = N // CH                   # 16 n-rows per chunk

_NC_CACHE = {}


def _build_nc():
    nc = bacc.Bacc()
    idx = nc.declare_dram_parameter("idx", [P, NBLK * K], mybir.dt.float32, isOutput=False)
    embrep = nc.declare_dram_parameter("embrep", [P, ND], mybir.dt.float32, isOutput=False)
    iota = nc.declare_dram_parameter("iota", [P, N], mybir.dt.float32, isOutput=False)
    out = nc.declare_dram_parameter("out", [ROWS, ND], mybir.dt.float32, isOutput=True)

    with TileContext(nc) as tc:
        with (
            tc.tile_pool(name="const", bufs=1) as cpool,
            tc.tile_pool(name="counts", bufs=NBLK) as hpool,
            tc.tile_pool(name="work", bufs=12) as wpool,
        ):
            # small inputs first so the first histogram can start immediately
            iota_sb = cpool.tile([P, N], mybir.dt.float32)
            nc.sync.dma_start(out=iota_sb, in_=iota[:, :])
            idx_sb = cpool.tile([P, NBLK * K], mybir.dt.float32)
            nc.sync.dma_start(out=idx_sb, in_=idx[:, :])
            # embedding replica loaded in chunks; chunk 0 lands first and
            # unblocks the first multiplies while the rest stream in during
            # the ramp, before the store stream saturates HBM.
            emb_sb = cpool.tile([P, ND], mybir.dt.float32)
            for c in range(CH):
                nc.sync.dma_start(
                    out=emb_sb[:, c * CW : (c + 1) * CW],
                    in_=embrep[:, c * CW : (c + 1) * CW],
                )

            def emit_hist(j, counts, pj):
                nc.vector.tensor_scalar(
                    out=counts[:pj],
                    in0=iota_sb[:pj],
                    scalar1=idx_sb[:pj, j * K : j * K + 1],
                    scalar2=None,
                    op0=mybir.AluOpType.is_equal,
                )
                for k in range(1, K):
                    nc.vector.scalar_tensor_tensor(
                        out=counts[:pj],
                        in0=iota_sb[:pj],
                        scalar=idx_sb[:pj, j * K + k : j * K + k + 1],
                        in1=counts[:pj],
                        op0=mybir.AluOpType.is_equal,
                        op1=mybir.AluOpType.add,
                    )

            def emit_mul(j, c, counts, pj):
                ot = wpool.tile([P, CW], mybir.dt.float32, tag="ot")
                nc.vector.tensor_tensor(
                    out=ot[:pj].rearrange("p (n d) -> p n d", d=D),
                    in0=counts[:pj, c * NCH : (c + 1) * NCH, None].broadcast_to(
                        [pj, NCH, D]
                    ),
                    in1=emb_sb[:pj, c * CW : (c + 1) * CW].rearrange(
                        "p (n d) -> p n d", d=D
                    ),
                    op=mybir.AluOpType.mult,
                )
                nc.sync.dma_start(
                    out=out[j * P : j * P + pj, c * CW : (c + 1) * CW],
                    in_=ot[:pj],
                )

            # chunk-major: the c=0 stripe (gated only on the small HBM
            # replica chunk) runs first, hiding the on-device broadcast
            # latency of chunks 1..3 behind ~40us of DVE work. Histograms
            # are interleaved into the first stripe so the first store
            # issues as early as possible.
            counts_tiles = [None] * NBLK
            for j in range(NBLK):
                pj = min(P, ROWS - j * P)
                counts = hpool.tile([P, N], mybir.dt.float32, tag="counts")
                counts_tiles[j] = counts
                emit_hist(j, counts, pj)
                emit_mul(j, 0, counts, pj)
            for c in range(1, CH):
                for j in range(NBLK):
                    pj = min(P, ROWS - j * P)
                    emit_mul(j, c, counts_tiles[j], pj)

    nc.finalize()
    return nc


def _get_nc():
    if "nc" not in _NC_CACHE:
        _NC_CACHE["nc"] = _build_nc()
    return _NC_CACHE["nc"]


def _prepare_in_maps(concepts, emb_table):
    concepts = np.asarray(concepts)
    emb = np.ascontiguousarray(np.asarray(emb_table, dtype=np.float32).reshape(1, ND))

    # per-core index shards, padded to NBLK*P rows, laid out [P, NBLK*K]
    conc = concepts.reshape(NCORES, ROWS, K).astype(np.float32)
    idx_pad = np.full((NCORES, NBLK * P, K), float(N), dtype=np.float32)
    idx_pad[:, :ROWS] = conc
    # [core, NBLK, P, K] -> [core, P, NBLK*K]
    idx_dev = np.ascontiguousarray(
        idx_pad.reshape(NCORES, NBLK, P, K).transpose(0, 2, 1, 3).reshape(NCORES, P, NBLK * K)
    )

    iota = np.ascontiguousarray(
        np.broadcast_to(np.arange(N, dtype=np.float32), (P, N))
    )
    embrep = np.ascontiguousarray(np.broadcast_to(emb, (P, ND)))
    return [
        {"idx": idx_dev[i], "embrep": embrep, "iota": iota}
        for i in range(NCORES)
    ]


def _run(concepts, emb_table, **spmd_kwargs):
    nc = _get_nc()
    in_maps = _prepare_in_maps(concepts, emb_table)
    res = run_bass_kernel_spmd(nc, in_maps, core_ids=list(range(NCORES)), **spmd_kwargs)
    out = np.concatenate(
        [res.results[i]["out"].reshape(B_LOC, S, N, D) for i in range(NCORES)],
        axis=0,
    )
    return out, res


def kernel(concepts, emb_table):
    out, _ = _run(concepts, emb_table)
    return out



# revision 7
# speedup vs baseline: 1.1913x; 1.1913x over previous
"""Trainium2 Bass kernel for nn_ConceptIntergation (histogram_binning).

Reference computation:
    counts[b,s,n] = sum_k one_hot(concepts[b,s,k], 129)[..., n]  (n < 128; 128 = padding)
    out[b,s,n,d]  = counts[b,s,n] * emb_table[n,d]

Strategy (data-parallel over batch, 8 cores):
  - Each core handles B_LOC=8 batches -> 1600 (b,s) rows. The kernel is
    HBM-write bound, so the output shard is written as FP16
    ([1600, 128*64] = 26 MB instead of 52 MB f32) and upcast to f32 on
    host. Max rel err of the fp16 path vs the f32 reference is ~7e-4,
    far inside the 2e-2 gate. All DVE ops run in fp16 for 2x rate.
  - Rows are processed in 128-row blocks (s on partitions). Histogram via
    iota-compare on DVE (tensor_scalar is_equal + scalar_tensor_tensor
    accumulate), then broadcast tensor_tensor multiplies produce
    [128, 2048] fp16 chunks = counts[:,n] * emb[n,d]; each chunk is a
    512 KB DMA store (contiguous 4 KB per partition row).
  - The embedding table is preloaded as an fp16 replica [128, 8192]
    (2 MB read), chunk by chunk so the first multiplies start early.
"""

import numpy as np

import concourse.bass as bass
import concourse.mybir as mybir
from concourse import bacc
from concourse.tile import TileContext
from concourse.bass_utils import run_bass_kernel_spmd

B, S, K = 64, 200, 4
N, D = 128, 64
ND = N * D                      # 8192
NCORES = 8
B_LOC = B // NCORES             # 8
ROWS = B_LOC * S                # 1600 (b,s) rows per core
P = 128
NBLK = (ROWS + P - 1) // P      # 13 (12 full + 1 of 64 rows)

CH = 4                          # emb-replica/mul/store chunks per block
CW = ND // CH                   # 2048 cols per chunk (= 16 n-rows)
NCH = N // CH                   # 16 n-rows per chunk

FP16 = mybir.dt.float16

_NC_CACHE = {}


def _build_nc():
    nc = bacc.Bacc()
    idx = nc.declare_dram_parameter("idx", [P, NBLK * K], mybir.dt.float32, isOutput=False)
    embrep = nc.declare_dram_parameter("embrep", [P, ND], FP16, isOutput=False)
    iota = nc.declare_dram_parameter("iota", [P, N], mybir.dt.float32, isOutput=False)
    out = nc.declare_dram_parameter("out", [ROWS, ND], FP16, isOutput=True)

    with TileContext(nc) as tc:
        with (
            tc.tile_pool(name="const", bufs=1) as cpool,
            tc.tile_pool(name="counts", bufs=NBLK) as hpool,
            tc.tile_pool(name="work", bufs=12) as wpool,
        ):
            # small inputs first so the first histogram can start immediately
            iota_sb = cpool.tile([P, N], mybir.dt.float32)
            nc.sync.dma_start(out=iota_sb, in_=iota[:, :])
            idx_sb = cpool.tile([P, NBLK * K], mybir.dt.float32)
            nc.sync.dma_start(out=idx_sb, in_=idx[:, :])
            # embedding replica loaded in chunks; chunk 0 lands first and
            # unblocks the first multiplies while the rest stream in during
            # the ramp, before the store stream saturates HBM.
            emb_sb = cpool.tile([P, ND], FP16)
            for c in range(CH):
                nc.sync.dma_start(
                    out=emb_sb[:, c * CW : (c + 1) * CW],
                    in_=embrep[:, c * CW : (c + 1) * CW],
                )

            def emit_hist(j, counts_f32, counts, pj):
                # histogram in f32 (is_equal requires f32 scalars), then a
                # cheap cast to fp16 on the otherwise-idle Act engine so the
                # big multiplies run all-16-bit at 2x DVE rate.
                nc.vector.tensor_scalar(
                    out=counts_f32[:pj],
                    in0=iota_sb[:pj],
                    scalar1=idx_sb[:pj, j * K : j * K + 1],
                    scalar2=None,
                    op0=mybir.AluOpType.is_equal,
                )
                for k in range(1, K):
                    nc.vector.scalar_tensor_tensor(
                        out=counts_f32[:pj],
                        in0=iota_sb[:pj],
                        scalar=idx_sb[:pj, j * K + k : j * K + k + 1],
                        in1=counts_f32[:pj],
                        op0=mybir.AluOpType.is_equal,
                        op1=mybir.AluOpType.add,
                    )
                nc.scalar.copy(out=counts[:pj], in_=counts_f32[:pj])

            def emit_mul(j, c, counts, pj):
                ot = wpool.tile([P, CW], FP16, tag="ot")
                nc.vector.tensor_tensor(
                    out=ot[:pj].rearrange("p (n d) -> p n d", d=D),
                    in0=counts[:pj, c * NCH : (c + 1) * NCH, None].broadcast_to(
                        [pj, NCH, D]
                    ),
                    in1=emb_sb[:pj, c * CW : (c + 1) * CW].rearrange(
                        "p (n d) -> p n d", d=D
                    ),
                    op=mybir.AluOpType.mult,
                )
                nc.sync.dma_start(
                    out=out[j * P : j * P + pj, c * CW : (c + 1) * CW],
                    in_=ot[:pj],
                )

            # chunk-major: the c=0 stripe (gated only on the small HBM
            # replica chunk) runs first. Histograms are interleaved into the
            # first stripe so the first store issues as early as possible.
            counts_tiles = [None] * NBLK
            for j in range(NBLK):
                pj = min(P, ROWS - j * P)
                counts_f32 = hpool.tile([P, N], mybir.dt.float32, tag="counts_f32")
                counts = hpool.tile([P, N], FP16, tag="counts")
                counts_tiles[j] = counts
                emit_hist(j, counts_f32, counts, pj)
                emit_mul(j, 0, counts, pj)
            for c in range(1, CH):
                for j in range(NBLK):
                    pj = min(P, ROWS - j * P)
                    emit_mul(j, c, counts_tiles[j], pj)

    nc.finalize()
    return nc


def _get_nc():
    if "nc" not in _NC_CACHE:
        _NC_CACHE["nc"] = _build_nc()
    return _NC_CACHE["nc"]


def _prepare_in_maps(concepts, emb_table):
    concepts = np.asarray(concepts)
    emb = np.ascontiguousarray(
        np.asarray(emb_table, dtype=np.float32).reshape(1, ND).astype(np.float16)
    )

    # per-core index shards, padded to NBLK*P rows, laid out [P, NBLK*K]
    conc = concepts.reshape(NCORES, ROWS, K).astype(np.float32)
    idx_pad = np.full((NCORES, NBLK * P, K), float(N), dtype=np.float32)
    idx_pad[:, :ROWS] = conc
    # [core, NBLK, P, K] -> [core, P, NBLK*K]
    idx_dev = np.ascontiguousarray(
        idx_pad.reshape(NCORES, NBLK, P, K).transpose(0, 2, 1, 3).reshape(NCORES, P, NBLK * K)
    )

    iota = np.ascontiguousarray(
        np.broadcast_to(np.arange(N, dtype=np.float32), (P, N))
    )
    embrep = np.ascontiguousarray(np.broadcast_to(emb, (P, ND)))
    return [
        {"idx": idx_dev[i], "embrep": embrep, "iota": iota}
        for i in range(NCORES)
    ]


def _run(concepts, emb_table, **spmd_kwargs):
    nc = _get_nc()
    in_maps = _prepare_in_maps(concepts, emb_table)
    res = run_bass_kernel_spmd(nc, in_maps, core_ids=list(range(NCORES)), **spmd_kwargs)
    out = np.concatenate(
        [
            res.results[i]["out"].astype(np.float32).reshape(B_LOC, S, N, D)
            for i in range(NCORES)
        ],
        axis=0,
    )
    return out, res


def kernel(concepts, emb_table):
    out, _ = _run(concepts, emb_table)
    return out


# revision 8
# speedup vs baseline: 1.4856x; 1.2470x over previous
"""Trainium2 Bass kernel for nn_ConceptIntergation (histogram_binning).

Reference computation:
    counts[b,s,n] = sum_k one_hot(concepts[b,s,k], 129)[..., n]  (n < 128; 128 = padding)
    out[b,s,n,d]  = counts[b,s,n] * emb_table[n,d]

Strategy (data-parallel over batch, 8 cores):
  - Each core handles B_LOC=8 batches -> 1600 (b,s) rows. The output shard
    is written as FP16 ([1600, 128*64] = 26 MB instead of 52 MB f32) and
    upcast to f32 on host; max rel err of the fp16 path is ~7e-4, far
    inside the tolerance. Store roofline ~73us/core at 358 GB/s.
  - The DVE cannot sustain the 13.1M-elem/core broadcast multiply
    (stride-0 operand forces 1x mode, ~115 G elem/s = 230 GB/s of fp16
    output < DMA floor), so the multiply runs on the idle TensorEngine:
        out_block[p, n*64+d] = sum_n' countsT[n', p] * W[n', n*64+d]
    with W the host-built block-diagonal [128, 8192] fp16 matrix
    W[n, n*64+d] = emb[n, d]. 208 matmuls of [128,128]@[128,512].
  - The histogram is computed directly transposed (countsT[n, p]) with
    the partition index as the is_equal scalar against a k-major
    replicated index tile: 1 tensor_scalar + 3 scalar_tensor_tensor of
    FD=1600 - ~4us total on DVE.
  - PSUM f32 -> SBUF fp16 downcast copies (the only remaining
    per-element engine work) alternate between Vector and Scalar
    engines (~123 + ~154 G elem/s combined), keeping both under the DMA
    store floor. Stores are 512 KB chunks, 4 KB contiguous per row.
"""

import numpy as np

import concourse.bass as bass
import concourse.mybir as mybir
from concourse import bacc
from concourse.tile import TileContext
from concourse.bass_utils import run_bass_kernel_spmd

B, S, K = 64, 200, 4
N, D = 128, 64
ND = N * D                      # 8192
NCORES = 8
B_LOC = B // NCORES             # 8
ROWS = B_LOC * S                # 1600 (b,s) rows per core
P = 128
NBLK = (ROWS + P - 1) // P      # 13 (12 full + 1 of 64 rows)

BIGC = 4                        # copy/store chunks per block
CW = ND // BIGC                 # 2048 cols per chunk
MMF = 512                       # matmul moving free size (one PSUM bank)
MM_PER_CHUNK = CW // MMF        # 4

FP16 = mybir.dt.float16
F32 = mybir.dt.float32

_NC_CACHE = {}


def _build_nc():
    nc = bacc.Bacc()
    idxrep = nc.declare_dram_parameter("idxrep", [P, K * ROWS], FP16, isOutput=False)
    wmat = nc.declare_dram_parameter("wmat", [P, ND], FP16, isOutput=False)
    iota_col = nc.declare_dram_parameter("iota_col", [P, 1], F32, isOutput=False)
    out = nc.declare_dram_parameter("out", [ROWS, ND], FP16, isOutput=True)

    with TileContext(nc) as tc:
        with (
            tc.tile_pool(name="const", bufs=1) as cpool,
            tc.tile_pool(name="work", bufs=12) as wpool,
            tc.tile_pool(name="psum", bufs=2, space="PSUM") as ppool,
        ):
            # small inputs first so the histogram can start immediately
            iota_sb = cpool.tile([P, 1], F32)
            nc.sync.dma_start(out=iota_sb, in_=iota_col[:, :])
            idxrep_sb = cpool.tile([P, K, ROWS], FP16)
            for k in range(K):
                nc.sync.dma_start(
                    out=idxrep_sb[:, k, :],
                    in_=idxrep[:, k * ROWS : (k + 1) * ROWS],
                )
            w_sb = cpool.tile([P, ND], FP16)
            for c in range(BIGC):
                nc.sync.dma_start(
                    out=w_sb[:, c * CW : (c + 1) * CW],
                    in_=wmat[:, c * CW : (c + 1) * CW],
                )

            # histogram, directly transposed: countsT[n, p] = #{k: idx[p,k]==n}
            countsT = cpool.tile([P, ROWS], FP16)
            nc.vector.tensor_scalar(
                out=countsT,
                in0=idxrep_sb[:, 0, :],
                scalar1=iota_sb,
                scalar2=None,
                op0=mybir.AluOpType.is_equal,
            )
            for k in range(1, K):
                nc.vector.scalar_tensor_tensor(
                    out=countsT,
                    in0=idxrep_sb[:, k, :],
                    scalar=iota_sb,
                    in1=countsT,
                    op0=mybir.AluOpType.is_equal,
                    op1=mybir.AluOpType.add,
                )

            # multiply on the PE; PSUM->SBUF downcast split across DVE/Act
            copy_i = 0
            for j in range(NBLK):
                pj = min(P, ROWS - j * P)
                stat = countsT[:, j * P : j * P + pj]
                for c in range(BIGC):
                    pt = ppool.tile([P, CW], F32, tag="pt")
                    for m in range(MM_PER_CHUNK):
                        lo = c * CW + m * MMF
                        nc.tensor.matmul(
                            pt[:pj, m * MMF : (m + 1) * MMF],
                            stat,
                            w_sb[:, lo : lo + MMF],
                            start=True,
                            stop=True,
                        )
                    ot = wpool.tile([P, CW], FP16, tag="ot")
                    if copy_i % 2 == 0:
                        nc.scalar.copy(out=ot[:pj], in_=pt[:pj])
                    else:
                        nc.vector.tensor_copy(out=ot[:pj], in_=pt[:pj])
                    copy_i += 1
                    nc.sync.dma_start(
                        out=out[j * P : j * P + pj, c * CW : (c + 1) * CW],
                        in_=ot[:pj],
                    )

    nc.finalize()
    return nc


def _get_nc():
    if "nc" not in _NC_CACHE:
        _NC_CACHE["nc"] = _build_nc()
    return _NC_CACHE["nc"]


def _prepare_in_maps(concepts, emb_table):
    concepts = np.asarray(concepts)
    emb = np.asarray(emb_table, dtype=np.float32).astype(np.float16)  # [N, D]

    # k-major replicated index shards: [core, P, K*ROWS]
    conc = concepts.reshape(NCORES, ROWS, K).astype(np.float16)
    idx_km = np.ascontiguousarray(conc.transpose(0, 2, 1))  # [core, K, ROWS]
    idxrep = np.ascontiguousarray(
        np.broadcast_to(idx_km.reshape(NCORES, 1, K * ROWS), (NCORES, P, K * ROWS))
    )

    # block-diagonal W: W[n, n*64+d] = emb[n, d]
    wmat = np.zeros((N, ND), dtype=np.float16)
    wmat[np.arange(N)[:, None], np.arange(N)[:, None] * D + np.arange(D)[None, :]] = emb
    wmat = np.ascontiguousarray(wmat)

    iota_col = np.ascontiguousarray(np.arange(P, dtype=np.float32).reshape(P, 1))
    return [
        {"idxrep": idxrep[i], "wmat": wmat, "iota_col": iota_col}
        for i in range(NCORES)
    ]


def _run(concepts, emb_table, **spmd_kwargs):
    nc = _get_nc()
    in_maps = _prepare_in_maps(concepts, emb_table)
    res = run_bass_kernel_spmd(nc, in_maps, core_ids=list(range(NCORES)), **spmd_kwargs)
    out = np.concatenate(
        [
            res.results[i]["out"].astype(np.float32).reshape(B_LOC, S, N, D)
            for i in range(NCORES)
        ],
        axis=0,
    )
    return out, res


def kernel(concepts, emb_table):
    out, _ = _run(concepts, emb_table)
    return out
